# revision 1
# baseline (speedup 1.0000x reference)
"""Trainium2 Bass kernel for nn_AttentionBlock (B=16, C=512, H=W=32, 8 heads).

Strategy: pure data-parallel over batch — 16 batches / 8 cores = 2 per core.
Each core computes, per batch:
  LayerNorm over channels (stats via ones-matmul partition reduction),
  QKV 1x1-conv GEMM (q,k in [c,t] layout packed per head-PAIR for
  row-tiled K=64 S-matmuls; v in [t,c] layout with interleaved ones
  columns so the AV matmul also produces softmax denominators),
  per-head attention (S = k^T q, exp on ScalarE with fused 1/8 scale,
  AV with fused denominator row, normalization via fast reciprocal),
  projection GEMM + residual.

All matmuls in bf16 (full-rate; fp32 accumulation in PSUM). All I/O fp32.
"""

import math

import numpy as np
import ml_dtypes

import concourse.bass as bass
import concourse.bacc as bacc
import concourse.tile as tile
from concourse import mybir
from concourse.bass_utils import run_bass_kernel_spmd

P = 128
C = 512
T = 1024
N_HEADS = 8
HD = 64
B = 16
N_CORES = 8
B_LOC = B // N_CORES  # batches per core
CCH = C // P  # channel chunks of 128
EPS = 1e-5

F32 = mybir.dt.float32
F32R = mybir.dt.float32r
BF16 = mybir.dt.bfloat16

HALVES = ((0, slice(0, 512)), (1, slice(512, 1024)))


def _interleave(*seqs):
    """Proportional merge of chunk lists (stable within each list)."""
    items = []
    for si, s in enumerate(seqs):
        n = max(len(s), 1)
        for i, c in enumerate(s):
            items.append(((i + 0.5) / n, si, c))
    items.sort(key=lambda t: (t[0], t[1]))
    return [c for _, _, c in items]


def _emit(tc, nc, pools, aps, dbg=None):
    fr = F32R
    mul = mybir.AluOpType.mult
    add = mybir.AluOpType.add
    sub = mybir.AluOpType.subtract

    x_d, wqk_d, wv_d, wp_d, bqk_d, bv_d, bp_d, out_d = aps
    (const, xpool, x2pool, xnpool, statp, xtmpp, qkpool, v2pool, hpool, expp, rdp, outp,
     psp, accp, drp) = pools

    # DRAM views
    xv = x_d.rearrange("b (cc p) t -> b p cc t", p=P)
    ov = out_d.rearrange("b (cc p) t -> b p cc t", p=P)

    # ---- constants (tiles allocated up front; DMAs emitted after the first
    # x DMA so the batch-0 input isn't queued behind 7MB of weights) ----
    wqk_sb = const.tile([P, CCH, 2 * C], BF16)
    wv_sb = const.tile([P, CCH, C], BF16)
    wp_sb = const.tile([P, CCH, C], BF16)
    bqk_sb = const.tile([P, 2 * C // P], F32)
    bp_sb = const.tile([P, CCH], F32)
    bv_b = const.tile([P, C], F32)
    ones_b = const.tile([P, P], BF16)
    eps_sb = const.tile([P, 1], F32)

    def emit_consts():
        nc.vector.memset(ones_b, 1.0)
        nc.vector.memset(eps_sb, EPS)
        nc.sync.dma_start(wqk_sb, wqk_d.rearrange("(cc p) o -> p cc o", p=P))
        nc.sync.dma_start(wv_sb, wv_d.rearrange("(cc p) o -> p cc o", p=P))
        nc.sync.dma_start(bqk_sb, bqk_d.rearrange("(o p) -> p o", p=P))
        nc.sync.dma_start(
            bv_b,
            bass.AP(tensor=bv_d.tensor, offset=bv_d.offset, ap=[[0, P]] + list(bv_d.ap)),
        )
        nc.sync.dma_start(bp_sb, bp_d.rearrange("(o p) -> p o", p=P))
        nc.sync.dma_start(wp_sb, wp_d.rearrange("(cc p) o -> p cc o", p=P))

    state = [dict() for _ in range(B_LOC)]

    # ---------------- phase A: LN + QKV ----------------
    def chunks_lnqkv(b):
        S = state[b]
        ch = []

        def c_load():
            S["x"] = xpool.tile([P, CCH, T], F32, tag="x", name="x_t")
            for cc in range(CCH):
                nc.sync.dma_start(S["x"][:, cc], xv[b, :, cc])
            S["x2"] = x2pool.tile([P, CCH, T], BF16, tag="x2", name="x2_t")
            S["xb"] = x2pool.tile([P, CCH, T], BF16, tag="xb", name="xb_t")
            S["v2"] = v2pool.tile([P, 8, N_HEADS * P], BF16, tag="v2", name="v2_t")
            nc.gpsimd.memset(S["v2"], 1.0)

        ch.append(c_load)

        def c_sq(cc):
            nc.vector.tensor_tensor(S["x2"][:, cc], S["x"][:, cc], S["x"][:, cc], mul)
            nc.scalar.activation(
                S["xb"][:, cc], S["x"][:, cc], mybir.ActivationFunctionType.Copy
            )

        for cc in range(CCH):
            ch.append(lambda cc=cc: c_sq(cc))

        def c_sums():
            S["muB"] = psp.tile([P, T], F32, tag="ps", name="ps_t")
            S["sqB"] = psp.tile([P, T], F32, tag="ps", name="ps_t")
            for _, hs in HALVES:
                for cc in range(CCH):
                    nc.tensor.matmul(
                        S["muB"][:, hs],
                        ones_b,
                        S["xb"][:, cc, hs],
                        start=(cc == 0),
                        stop=(cc == CCH - 1),
                    )
            for _, hs in HALVES:
                for cc in range(CCH):
                    nc.tensor.matmul(
                        S["sqB"][:, hs],
                        ones_b,
                        S["x2"][:, cc, hs],
                        start=(cc == 0),
                        stop=(cc == CCH - 1),
                    )

        ch.append(c_sums)

        def c_stats():
            m = statp.tile([P, T], F32, tag="stat", name="stat_t")
            nc.vector.tensor_scalar_mul(m, S["muB"], 1.0 / C)
            var = statp.tile([P, T], F32, tag="stat", name="stat_t")
            nc.vector.tensor_tensor(var, m, m, mul)
            nc.vector.scalar_tensor_tensor(var, S["sqB"], 1.0 / C, var, mul, sub)
            nc.scalar.activation(var, var, mybir.ActivationFunctionType.Sqrt, bias=eps_sb, scale=1.0)
            rstd = statp.tile([P, T], F32, tag="stat", name="stat_t")
            nc.vector.reciprocal_approx_fast(rstd, var)
            S["m"], S["rstd"] = m, rstd
            S["xn"] = xnpool.tile([P, CCH, T], BF16, tag="xn", name="xn_t")

        ch.append(c_stats)

        def c_xn(cc):
            t = xtmpp.tile([P, T], F32, tag="xtmp", name="xtmp_t")
            nc.vector.tensor_tensor(t, S["x"][:, cc], S["m"], sub)
            nc.vector.tensor_tensor(S["xn"][:, cc], t, S["rstd"], mul)

        for cc in range(CCH):
            ch.append(lambda cc=cc: c_xn(cc))

        def c_dbg_a():
            if dbg is not None and b == 0:
                nc.sync.dma_start(dbg["stats"][0], S["m"])
                nc.sync.dma_start(dbg["stats"][1], S["rstd"])
                nc.sync.dma_start(dbg["xn"], S["xn"])

        ch.append(c_dbg_a)

        def c_qkgen(ot):
            if "qk" not in S:
                S["qk"] = qkpool.tile([P, 8, T], BF16, tag="qk", name="qk_t")
            ps = psp.tile([P, T], F32, tag="ps", name="ps_t")
            for _, hs in HALVES:
                for cc in range(CCH):
                    nc.tensor.matmul(
                        ps[:, hs],
                        wqk_sb[:, cc, ot * P : (ot + 1) * P],
                        S["xn"][:, cc, hs],
                        start=(cc == 0),
                        stop=(cc == CCH - 1),
                    )
            nc.vector.tensor_scalar_add(S["qk"][:, ot], ps, bqk_sb[:, ot : ot + 1])

        for ot in range(8):
            ch.append(lambda ot=ot: c_qkgen(ot))

        def c_vgen(st):
            ps = psp.tile([P, T], F32, tag="ps", name="ps_t")
            tsl = slice(st * P, (st + 1) * P)
            for cc in range(CCH):
                nc.tensor.matmul(
                    ps[:, 0:512],
                    S["xn"][:, cc, tsl],
                    wv_sb[:, cc, :],
                    start=(cc == 0),
                    stop=(cc == CCH - 1),
                )
            pr = ps[:, 0:512].rearrange("p (h c) -> p h c", c=HD)
            bvr = bv_b.rearrange("p (h c) -> p h c", c=HD)
            v2r = S["v2"].rearrange("p st (h c) -> p st h c", c=P)
            # even heads: v data in cols 0:64 (ones stay in 64:128)
            nc.vector.tensor_tensor(v2r[:, st, 0::2, 0:HD], pr[:, 0::2], bvr[:, 0::2], add)
            # odd heads: v data in cols 64:128 (ones stay in 0:64)
            nc.vector.tensor_tensor(v2r[:, st, 1::2, HD:P], pr[:, 1::2], bvr[:, 1::2], add)

        for st in range(8):
            ch.append(lambda st=st: c_vgen(st))

        def c_dbg_b():
            if dbg is not None and b == 0:
                nc.sync.dma_start(dbg["qk"], S["qk"])
                nc.sync.dma_start(dbg["v2"], S["v2"])

        ch.append(c_dbg_b)
        return ch

    # ---------------- phase B: attention ----------------
    def chunks_attn(b):
        S = state[b]
        ch = []

        def c_pair_start(pc):
            S[("acc", pc)] = {}
            S[("exp", pc)] = {}
            for h01 in (0, 1):
                for hf, _ in HALVES:
                    S[("acc", pc)][(h01, hf)] = accp.tile([P, 512], F32, tag="acc", name="acc_t")

        def fin_head(pc, h01):
            head = 2 * pc + h01
            data = slice(HD * h01, HD * h01 + HD)
            dnm = slice(HD * (1 - h01), HD * (1 - h01) + HD)
            cch = head // 2
            if "h" not in S:
                S["h"] = hpool.tile([P, CCH, T], BF16, tag="h", name="h_t")
            for hf, hs in HALVES:
                o_ps = S[("acc", pc)][(h01, hf)]
                rd = rdp.tile([P, 512], F32, tag="rd", name="rd_t")
                if h01 == 1:
                    nc.vector.reciprocal_approx_fast(rd[dnm], o_ps[dnm])
                else:
                    nc.vector.tensor_copy(rd[dnm], o_ps[dnm])
                sc = drp.tile([1, 512], F32, tag="rdd", name="rdd_t")
                nc.sync.dma_start(sc, rd[dnm.start : dnm.start + 1, :])
                bcast = bass.AP(
                    tensor=sc.tensor, offset=sc.offset,
                    ap=[[0, HD]] + [list(a) for a in sc.ap[1:]],
                )
                nc.sync.dma_start(rd[data], bcast)
                if h01 == 0:
                    nc.vector.reciprocal_approx_fast(rd[data], rd[data])
                if dbg is not None and b == 0 and pc < 2 and hf == 0:
                    tmp = outp.tile([P, 512], F32, tag="out", name="dbgcp_t")
                    nc.vector.tensor_copy(tmp, o_ps)
                    nc.sync.dma_start(dbg["acc"][2 * pc + h01], tmp)
                    nc.sync.dma_start(dbg["rd"][2 * pc + h01], rd)
                nc.vector.tensor_tensor(S["h"][data, cch, hs], o_ps[data], rd[data], mul)

        def c_st(pc, st):
            qt = S["qk"][:, 2 * pc]
            kt = S["qk"][:, 2 * pc + 1]
            tsl = slice(st * P, (st + 1) * P)
            pss = {}
            for h01 in (0, 1):
                pss[h01] = psp.tile([P, T], F32, tag="ps", name="ps_t")
            for _, hs in HALVES:
                for h01 in (0, 1):
                    bb = slice(HD * h01, HD * h01 + HD)
                    nc.tensor.matmul(
                        pss[h01][:, hs], kt[bb, tsl], qt[bb, hs], start=True, stop=True
                    )
            for h01 in (0, 1):
                e = expp.tile([P, T], BF16, tag="exp", name="exp_t")
                nc.scalar.activation(
                    e, pss[h01], mybir.ActivationFunctionType.Exp, scale=0.125
                )
                S[("exp", pc)][h01] = e
                if dbg is not None and b == 0 and pc == 0 and st == 0:
                    nc.sync.dma_start(dbg["exp"][h01], e)
                head = 2 * pc + h01
                for hf, hs in HALVES:
                    nc.tensor.matmul(
                        S[("acc", pc)][(h01, hf)],
                        S["v2"][:, st, head * P : (head + 1) * P],
                        e[:, hs],
                        start=(st == 0),
                        stop=(st == 7),
                    )
                if st == 7:
                    fin_head(pc, h01)

        def c_fin(pc):
            pass  # finalize now happens per-head inside the st==7 step

        for pc in range(4):
            ch.append(lambda pc=pc: c_pair_start(pc))
            for st in range(8):
                ch.append(lambda pc=pc, st=st: c_st(pc, st))
            ch.append(lambda pc=pc: c_fin(pc))
        return ch

    # ---------------- phase C: proj + residual + out ----------------
    def chunks_proj(b):
        S = state[b]
        ch = []

        def c_proj(ot, hf, hs):
            ps = psp.tile([P, T], F32, tag="ps", name="ps_t")
            for cc in range(CCH):
                nc.tensor.matmul(
                    ps[:, 0:512],
                    wp_sb[:, cc, ot * P : (ot + 1) * P],
                    S["h"][:, cc, hs],
                    start=(cc == 0),
                    stop=(cc == CCH - 1),
                )
            o_t = outp.tile([P, 512], F32, tag="out", name="out_t")
            nc.vector.scalar_tensor_tensor(
                o_t, ps[:, 0:512], bp_sb[:, ot : ot + 1], S["x"][:, ot, hs], add, add
            )
            nc.sync.dma_start(ov[b, :, ot, hs], o_t)

        def c_dbg_h():
            if dbg is not None and b == 0:
                nc.sync.dma_start(dbg["h"], S["h"])

        ch.append(c_dbg_h)
        for ot in range(CCH):
            for hf, hs in HALVES:
                ch.append(lambda ot=ot, hf=hf, hs=hs: c_proj(ot, hf, hs))
        return ch

    # ---------------- emission schedule (software pipeline) ----------------
    a0 = chunks_lnqkv(0)
    a0[0]()
    emit_consts()
    for c in a0[1:]:
        c()
    # Software pipeline: batch-0 attention (ScalarE-heavy) carries batch-1
    # LN/QKV (PE-heavy); batch-1 attention carries batch-0 projection.
    for c in _interleave(chunks_attn(0), chunks_lnqkv(1)):
        c()
    b1 = chunks_attn(1)
    c0 = chunks_proj(0)
    # b1 chunk layout: per pair [start, st0..st7, fin] = 10 chunks
    out_order = []
    ci = 0
    for i, c in enumerate(b1):
        out_order.append(c)
        if i % 10 == 9 and ci < len(c0):  # after each pair finalize
            out_order.extend(c0[ci : ci + 2])
            ci += 2
    out_order.extend(c0[ci:])
    for c in out_order:
        c()
    for c in chunks_proj(1):
        c()


def build_nc(debug_taps=False):
    nc = bacc.Bacc("TRN2", num_devices=N_CORES, debug=False)
    x = nc.declare_dram_parameter("x", [B_LOC, C, T], F32, isOutput=False)
    wqk = nc.declare_dram_parameter("w_qkT", [C, 2 * C], BF16, isOutput=False)
    wv = nc.declare_dram_parameter("w_vT", [C, C], BF16, isOutput=False)
    wp = nc.declare_dram_parameter("w_projT", [C, C], BF16, isOutput=False)
    bqk = nc.declare_dram_parameter("b_qk", [2 * C], F32, isOutput=False)
    bv = nc.declare_dram_parameter("b_v", [C], F32, isOutput=False)
    bp = nc.declare_dram_parameter("b_proj", [C], F32, isOutput=False)
    out = nc.declare_dram_parameter("out", [B_LOC, C, T], F32, isOutput=True)
    aps = (x.ap(), wqk.ap(), wv.ap(), wp.ap(), bqk.ap(), bv.ap(), bp.ap(), out.ap())
    dbg = None
    if debug_taps:
        dbg = {
            "stats": nc.declare_dram_parameter("dbg_stats", [2, P, T], F32, isOutput=True).ap(),
            "xn": nc.declare_dram_parameter("dbg_xn", [P, CCH, T], BF16, isOutput=True).ap(),
            "qk": nc.declare_dram_parameter("dbg_qk", [P, 8, T], BF16, isOutput=True).ap(),
            "v2": nc.declare_dram_parameter("dbg_v2", [P, 8, N_HEADS * P], BF16, isOutput=True).ap(),
            "exp": nc.declare_dram_parameter("dbg_exp", [2, P, T], BF16, isOutput=True).ap(),
            "acc": nc.declare_dram_parameter("dbg_acc", [4, P, 512], F32, isOutput=True).ap(),
            "rd": nc.declare_dram_parameter("dbg_rd", [4, P, 512], F32, isOutput=True).ap(),
            "h": nc.declare_dram_parameter("dbg_h", [P, CCH, T], BF16, isOutput=True).ap(),
        }

    with tile.TileContext(nc) as tc:
        import contextlib

        with contextlib.ExitStack() as ctx:
            pools = (
                ctx.enter_context(tc.tile_pool(name="const", bufs=1)),
                ctx.enter_context(tc.tile_pool(name="x", bufs=2)),
                ctx.enter_context(tc.tile_pool(name="x2", bufs=1)),
                ctx.enter_context(tc.tile_pool(name="xn", bufs=2)),
                ctx.enter_context(tc.tile_pool(name="stat", bufs=3)),
                ctx.enter_context(tc.tile_pool(name="xtmp", bufs=1)),
                ctx.enter_context(tc.tile_pool(name="qk", bufs=2)),
                ctx.enter_context(tc.tile_pool(name="v2", bufs=2)),
                ctx.enter_context(tc.tile_pool(name="h", bufs=2)),
                ctx.enter_context(tc.tile_pool(name="exp", bufs=4)),
                ctx.enter_context(tc.tile_pool(name="rd", bufs=4)),
                ctx.enter_context(tc.tile_pool(name="out", bufs=2)),
                ctx.enter_context(tc.tile_pool(name="ps", bufs=2, space="PSUM")),
                ctx.enter_context(tc.tile_pool(name="acc", bufs=4, space="PSUM")),
                ctx.enter_context(tc.tile_pool(name="drd", bufs=8, space="DRAM")),
            )
            _emit(tc, nc, pools, aps, dbg)
    nc.compile()
    return nc


def _host_prep(w_qkv, b_qkv, w_proj, b_proj):
    rows = np.arange(3 * C).reshape(N_HEADS, 3, HD)
    qk_order = []
    for pc in range(4):
        qk_order += list(rows[2 * pc, 0]) + list(rows[2 * pc + 1, 0])
        qk_order += list(rows[2 * pc, 1]) + list(rows[2 * pc + 1, 1])
    qk_order = np.array(qk_order)
    v_order = rows[:, 2, :].reshape(-1)
    prep = {
        "w_qkT": np.ascontiguousarray(w_qkv[qk_order].T).astype(ml_dtypes.bfloat16),
        "w_vT": np.ascontiguousarray(w_qkv[v_order].T).astype(ml_dtypes.bfloat16),
        "w_projT": np.ascontiguousarray(w_proj.T).astype(ml_dtypes.bfloat16),
        "b_qk": np.ascontiguousarray(b_qkv[qk_order]).astype(np.float32),
        "b_v": np.ascontiguousarray(b_qkv[v_order]).astype(np.float32),
        "b_proj": np.ascontiguousarray(b_proj).astype(np.float32),
    }
    return prep


_NC = None


def kernel(x, emb, w_qkv, b_qkv, w_proj, b_proj):
    global _NC
    x = np.asarray(x, dtype=np.float32)
    b, c, hh, ww = x.shape
    assert (b, c, hh * ww) == (B, C, T)
    prep = _host_prep(
        np.asarray(w_qkv, np.float32),
        np.asarray(b_qkv, np.float32),
        np.asarray(w_proj, np.float32),
        np.asarray(b_proj, np.float32),
    )
    xf = x.reshape(B, C, T)
    if _NC is None:
        _NC = build_nc()
    in_maps = []
    for core in range(N_CORES):
        m = dict(prep)
        m["x"] = np.ascontiguousarray(xf[core * B_LOC : (core + 1) * B_LOC])
        in_maps.append(m)
    res = run_bass_kernel_spmd(_NC, in_maps, core_ids=list(range(N_CORES)), trace=False)
    out = np.concatenate([res.results[i]["out"] for i in range(N_CORES)], axis=0)
    return out.reshape(B, C, hh, ww).astype(np.float32)

